# revision 15
# baseline (speedup 1.0000x reference)
"""Cross-attention Trainium2 kernel, 8-core data-parallel.

Problem (hardcoded): B=4, NQ=4096, NK=1024, QD=1024, CD=768, H=16, HD=64.
  out = softmax((x@Wq) @ (ctx@Wk)^T / sqrt(HD)) @ (ctx@Wv) @ Wo + bo

Sharding: pure data-parallel. 8 cores = 4 batches x 2 NQ-halves of 2048
query rows. Each core redundantly computes K/V projections for its batch
(cheap) and needs no collectives.

v2 design (from perfetto analysis of v1):
 - The scalar ACT engine (exp over 33.5M scores/core at 1 elem/cycle/lane
   @1.2GHz + ~300cyc/instr overhead) is the binding engine of the
   attention phase.  Design goal: ACT 100% duty, everything else hidden.
 - exp instructions enlarged to N=2048/1024 via PSUM staging slots
   A=[128,2,1024] f32 (4 banks) + B=[128,1024] (2 banks); per head-pair
   the 8 key-chunks map kc{0,1}->A, kc2->B, kc{3,4}->A, kc5->B,
   kc{6,7}->A giving 5 exps (3x2048 + 2x1024) instead of 8x1024.
 - One shared PSUM accumulator ring ACC=[128,2,512] f32 (2 banks) serves
   attn@V groups, out-projection, Q/K/V-projection groups.
 - 4 query-tiles of 512.  PE stream per (tile, head-pair): score
   pair-matmuls (tile_position row-split => the two 64-contraction
   matmuls run concurrently), the PREVIOUS head-pair's attn@V (its exp
   outputs are complete; etp is a 2-deep ring over head-pairs), and
   filler matmul groups (V-projection during tile0, out-projection of
   tile t-1, Q-projection of tile t+1) slotted between score pairs so
   the PE always has runnable work while exp drains.
 - Scalar engine does nothing but exp during attention; PSUM->SBUF
   copies go to vector/gpsimd.
"""

import numpy as np

B, NQ, NK = 4, 4096, 1024
QD, CD, H = 1024, 768, 16
HD = QD // H
SCALE = HD ** -0.5
NQL = NQ // 2          # query rows per core
N_CORES = 8
W = 512                # query tile width
NT = NQL // W          # 4 query tiles
KC_Q = QD // 128       # 8
KC_C = CD // 128       # 6
NKC = NK // 128        # 8
HP = H // 2            # 8 head pairs


def build_bass():
    """Build the per-core Bass graph (SPMD, identical on all 8 cores)."""
    import concourse.bass as bass
    import concourse.tile as tile
    from concourse import bacc, mybir

    f32 = mybir.dt.float32
    bf16 = mybir.dt.bfloat16
    EXP = mybir.ActivationFunctionType.Exp

    nc = bacc.Bacc()

    xT_h = nc.dram_tensor("xT", (QD, NQL), bf16, kind="ExternalInput")
    ctxT_h = nc.dram_tensor("ctxT", (CD, NK), bf16, kind="ExternalInput")
    wq_h = nc.dram_tensor("wq", (QD, QD), bf16, kind="ExternalInput")
    wk_h = nc.dram_tensor("wk", (CD, QD), bf16, kind="ExternalInput")
    wv_h = nc.dram_tensor("wv", (CD, QD), bf16, kind="ExternalInput")
    wo_h = nc.dram_tensor("wo", (QD, QD), bf16, kind="ExternalInput")
    bo_h = nc.dram_tensor("bo", (1, QD), bf16, kind="ExternalInput")
    eye_h = nc.dram_tensor("eye", (128, 128), bf16, kind="ExternalInput")
    out_h = nc.dram_tensor("out", (NQL, QD), f32, kind="ExternalOutput")

    xT_d = xT_h[:].rearrange("(c p) n -> p c n", p=128)       # [128, 8, 2048]
    ctxT_d = ctxT_h[:].rearrange("(c p) n -> p c n", p=128)   # [128, 6, 1024]
    wq_d = wq_h[:].rearrange("(c p) m -> p c m", p=128)
    wk_d = wk_h[:].rearrange("(c p) m -> p c m", p=128)
    wv_d = wv_h[:].rearrange("(c p) m -> p c m", p=128)
    wo_d = wo_h[:].rearrange("(c p) m -> p c m", p=128)
    out_d = out_h[:].rearrange("(t p) n -> p t n", p=128)     # [128, 16, 1024]

    with tile.TileContext(nc) as tc:
        _cms = []

        def open_pool(**kw):
            cm = tc.tile_pool(**kw)
            _cms.append(cm)
            return cm.__enter__()

        pp = open_pool(name="persist", bufs=1)
        # ---- persistent SBUF tiles
        qt_sb = pp.tile([128, KC_Q, 2, W], bf16)       # QT ring (per-tile)
        kt_sb = pp.tile([128, KC_Q, NK], bf16)         # KT   2 MB
        vp_sb = pp.tile([128, NKC, H, HD + 1], bf16)   # V'   2.08 MB
        attnT_sb = pp.tile([128, KC_Q, 2, W], bf16)    # attn ring (per-tile)
        wo_sb = pp.tile([128, KC_Q, QD], bf16)         # Wo   2 MB
        wq_sb = pp.tile([128, KC_Q, QD], bf16)         # Wq   2 MB
        # exp outputs: ring of 2 hp-buffers: slot index = (hp%2)*8 + kc
        etp = pp.tile([128, 16, 2, W], bf16)           # 4.2 MB
        bo_bc = pp.tile([128, QD], bf16)
        ones_sb = pp.tile([1, 128], bf16)
        eye_sb = pp.tile([128, 128], bf16)

        # ---- PSUM (8 banks): A 4 + Bp 2 + ACC ring 2
        psS = open_pool(name="psS", bufs=1, space=bass.MemorySpace.PSUM)
        A = psS.tile([128, 2, 1024], f32)
        Bp = psS.tile([128, 1024], f32)
        accp = open_pool(name="accp", bufs=2, space=bass.MemorySpace.PSUM)

        # ---- small SBUF pools
        py = open_pool(name="yout", bufs=3)
        prs = open_pool(name="rsmall", bufs=2)
        pxq = open_pool(name="xq", bufs=2)
        pkv = open_pool(name="kvin", bufs=1)

        ctxT_sb = pkv.tile([128, KC_C, NK], bf16)      # 1.5 MB
        wk_sb = pkv.tile([128, KC_C, QD], bf16)        # 1.5 MB
        wv_sb = pkv.tile([128, KC_C, QD], bf16)        # 1.5 MB
        bo_sb = pkv.tile([1, QD], bf16)

        # copy engines for PSUM->SBUF moves; scalar only in the prefix
        _cp_idx = [0]

        def copy_out(dst, src, in_attention):
            # gpsimd cannot read PSUM; scalar must stay free during attention
            engs = (nc.vector,) if in_attention else (nc.vector, nc.scalar)
            eng = engs[_cp_idx[0] % len(engs)]
            _cp_idx[0] += 1
            if eng is nc.scalar:
                eng.copy(dst, src)
            else:
                eng.tensor_copy(dst, src)

        # ---------------- prefix ----------------
        nc.sync.dma_start(ctxT_sb[:, :, 0:512], ctxT_d[:, :, 0:512])
        nc.sync.dma_start(wk_sb[:, :, 0:256], wk_d[:, :, 0:256])
        nc.sync.dma_start(bo_sb[:], bo_h[:])
        nc.sync.dma_start(ctxT_sb[:, :, 512:1024], ctxT_d[:, :, 512:1024])
        nc.sync.dma_start(wk_sb[:, :, 256:1024], wk_d[:, :, 256:1024])

        nc.sync.dma_start(eye_sb[:], eye_h[:])
        xt0 = pxq.tile([128, KC_Q, W], bf16, tag="xt", name="xt0")

        nc.vector.memset(ones_sb[:], 1.0)
        nc.vector.memset(vp_sb[:, :, :, HD], 1.0)    # ones column per head
        for no in range(2):
            ps = accp.tile([128, 512], f32, tag="acc", name="psb")
            nc.tensor.matmul(ps[:], ones_sb[:],
                             bo_sb[0:1, no * 512:(no + 1) * 512],
                             start=True, stop=True)
            nc.scalar.copy(bo_bc[:, no * 512:(no + 1) * 512], ps[:])

        # ---- K projection (all keys; needed before any scores)
        for mo in range(KC_Q):
            for nk in range(2):
                ps = accp.tile([128, 512], f32, tag="acc", name="psk")
                for c in range(KC_C):
                    nc.tensor.matmul(
                        ps[:],
                        wk_sb[:, c, mo * 128:(mo + 1) * 128],
                        ctxT_sb[:, c, nk * 512:(nk + 1) * 512],
                        start=(c == 0), stop=(c == KC_C - 1),
                    )
                copy_out(kt_sb[:, mo, nk * 512:(nk + 1) * 512], ps[:], False)
            if mo == 0:
                nc.sync.dma_start(wv_sb[:], wv_d)
                nc.sync.dma_start(xt0[:], xT_d[:, :, 0:W])
                nc.sync.dma_start(wq_sb[:], wq_d)

        def v_proj(ko, nv, in_attention):
            ps = accp.tile([128, 512], f32, tag="acc", name="psv")
            for c in range(KC_C):
                nc.tensor.matmul(
                    ps[:],
                    ctxT_sb[:, c, ko * 128:(ko + 1) * 128],
                    wv_sb[:, c, nv * 512:(nv + 1) * 512],
                    start=(c == 0), stop=(c == KC_C - 1),
                )
            copy_out(
                vp_sb[:, ko, nv * 8:(nv + 1) * 8, 0:HD],
                ps[:].rearrange("p (h d) -> p h d", h=8), in_attention)

        # ---- V projection for heads 0-7 (needed by tile0's early attn@V)
        for ko in range(NKC):
            v_proj(ko, 0, False)

        # ---- Q projection for tile 0
        for mo in range(KC_Q):
            ps = accp.tile([128, 512], f32, tag="acc", name="psq")
            for c in range(KC_Q):
                nc.tensor.matmul(
                    ps[:],
                    wq_sb[:, c, mo * 128:(mo + 1) * 128],
                    xt0[:, c, :],
                    start=(c == 0), stop=(c == KC_Q - 1),
                )
            copy_out(qt_sb[:, mo, 0, :], ps[:], False)

        nc.sync.dma_start(wo_sb[:], wo_d)

        # ---------- filler generators (yield once per matmul group) ----
        # V heads 8-15: drained during tile0 hp0-3, strictly before any
        # attn@V of hp>=4 (which is emitted at hp>=5) reads them.
        def v_group():
            for ko in range(NKC):
                v_proj(ko, 1, True)
                yield

        def q_group(t, xt):
            for mo in range(KC_Q):
                ps = accp.tile([128, 512], f32, tag="acc", name="psq2")
                for c in range(KC_Q):
                    nc.tensor.matmul(
                        ps[:],
                        wq_sb[:, c, mo * 128:(mo + 1) * 128],
                        xt[:, c, :],
                        start=(c == 0), stop=(c == KC_Q - 1),
                    )
                copy_out(qt_sb[:, mo, t % 2, :], ps[:], True)
                yield

        def y_group(t):
            for lo in range(4):
                mo = t * 4 + lo
                for no in range(2):
                    ps = accp.tile([128, 512], f32, tag="acc", name="psy")
                    for c in range(KC_Q):
                        nc.tensor.matmul(
                            ps[:],
                            attnT_sb[:, c, t % 2, lo * 128:(lo + 1) * 128],
                            wo_sb[:, c, no * 512:(no + 1) * 512],
                            start=(c == 0), stop=(c == KC_Q - 1),
                        )
                    y = py.tile([128, 512], f32, tag="y")
                    nc.vector.tensor_add(
                        y[:], ps[:], bo_bc[:, no * 512:(no + 1) * 512])
                    nc.sync.dma_start(
                        out_d[:, mo, no * 512:(no + 1) * 512], y[:])
                    yield

        # ---------------- main attention loop ----------------
        pst = open_pool(name="stg", bufs=12)
        _stages = {}

        def attn_vO(t, hp, qcs):
            """attn@V in O[q,d] form + normalize into SBUF stage tiles.

            One PSUM slot holds both heads: O[:,0:65]=h0 (col 64 = softmax
            denominator), O[:,65:130]=h1.  Normalize with a per-partition
            reciprocal + tensor_scalar into stage[q,d] (bf16).  The PE
            transpose back to attnT's [d,q] layout happens one head-pair
            later (attn_vT) so the PE never waits on this DVE chain.
            """
            r = hp % 2
            for qc in qcs:
                O = accp.tile([128, 512], f32, tag="acc", name="O")
                for h_i in (0, 1):
                    h = 2 * hp + h_i
                    dst = O[:, h_i * 65:h_i * 65 + 65]
                    for kc in range(NKC):
                        nc.tensor.matmul(
                            dst,
                            etp[:, r * 8 + kc, h_i, qc * 128:(qc + 1) * 128],
                            vp_sb[:, kc, h, :],
                            start=(kc == 0), stop=(kc == NKC - 1),
                        )
                stage = pst.tile([128, 128], bf16, tag="st")
                for h_i in (0, 1):
                    rcp = prs.tile([128, 1], f32, tag="rcp")
                    nc.vector.reciprocal_approx_fast(
                        rcp[:], O[:, h_i * 65 + 64:h_i * 65 + 65])
                    nc.vector.tensor_scalar_mul(
                        stage[:, h_i * 64:(h_i + 1) * 64],
                        O[:, h_i * 65:h_i * 65 + 64], rcp[:])
                _stages[(t, hp, qc)] = stage

        def attn_vT(t, hp, qcs):
            for qc in qcs:
                stage = _stages.pop((t, hp, qc))
                tp = accp.tile([128, 512], f32, tag="acc", name="tp")
                tp16 = tp[:, 0:64].bitcast(bf16)
                nc.tensor.transpose(tp16, stage[:], eye_sb[:])
                nc.vector.tensor_copy(
                    attnT_sb[:, hp, t % 2, qc * 128:(qc + 1) * 128], tp16)

        pending = None          # (t, hp) owed attn@V O/normalize
        pending2 = None         # (t, hp) owed transposes
        fillers = []

        def take_filler(n):
            # drain sequentially: first generator until exhausted, then next
            for _ in range(n):
                while fillers:
                    if next(fillers[0], "DONE") == "DONE":
                        fillers.pop(0)
                        continue
                    break

        # y(t-1) must not drain at hp0: its c=7 matmul needs the transpose
        # of (t-1, hp7) which is only emitted at hp1.
        N_FILL = [0, 2, 2, 3, 3, 2, 2, 2]

        for t in range(NT):
            if t == 0:
                fillers.append(v_group())
            else:
                fillers.append(y_group(t - 1))
            if t + 1 < NT:
                xt = pxq.tile([128, KC_Q, W], bf16, tag="xt",
                              name=f"xt{t + 1}")
                nc.sync.dma_start(xt[:], xT_d[:, :, (t + 1) * W:(t + 2) * W])
                fillers.append(q_group(t + 1, xt))

            for hp in range(HP):
                r = hp % 2

                def spair(kc, dst_h0, dst_h1):
                    ks = slice(kc * 128, (kc + 1) * 128)
                    nc.tensor.matmul(
                        dst_h0, kt_sb[0:64, hp, ks],
                        qt_sb[0:64, hp, t % 2, :],
                        start=True, stop=True, tile_position=(0, 0),
                    )
                    nc.tensor.matmul(
                        dst_h1, kt_sb[64:128, hp, ks],
                        qt_sb[64:128, hp, t % 2, :],
                        start=True, stop=True, tile_position=(64, 0),
                    )

                def exp_a(kc):
                    nc.scalar.activation(
                        etp[:, r * 8 + kc:r * 8 + kc + 2, :, :],
                        A[:], EXP, scale=SCALE)

                def exp_b(kc):
                    nc.scalar.activation(
                        etp[:, r * 8 + kc:r * 8 + kc + 1, :, :],
                        Bp[:], EXP, scale=SCALE)

                # kc pattern: {0,1}->A, 2->B, {3,4}->A, 5->B, {6,7}->A
                spair(0, A[:, 0, 0:512], A[:, 0, 512:1024])
                spair(1, A[:, 1, 0:512], A[:, 1, 512:1024])
                exp_a(0)
                spair(2, Bp[:, 0:512], Bp[:, 512:1024])
                exp_b(2)
                if pending2 is not None:
                    attn_vT(*pending2, (0, 1, 2, 3))
                if pending is not None:
                    attn_vO(*pending, (0, 1))
                spair(3, A[:, 0, 0:512], A[:, 0, 512:1024])
                spair(4, A[:, 1, 0:512], A[:, 1, 512:1024])
                exp_a(3)
                if pending is not None:
                    attn_vO(*pending, (2,))
                spair(5, Bp[:, 0:512], Bp[:, 512:1024])
                exp_b(5)
                if pending is not None:
                    attn_vO(*pending, (3,))
                spair(6, A[:, 0, 0:512], A[:, 0, 512:1024])
                spair(7, A[:, 1, 0:512], A[:, 1, 512:1024])
                exp_a(6)
                take_filler(N_FILL[hp])
                pending2 = pending
                pending = (t, hp)

        # drain: last two head-pairs' attn@V stages + last y tile
        attn_vT(*pending2, (0, 1))
        attn_vO(*pending, (0, 1, 2, 3))
        attn_vT(*pending2, (2, 3))
        take_filler(1000)
        attn_vT(*pending, (0, 1, 2, 3))
        for _ in y_group(NT - 1):
            pass

        for cm in reversed(_cms):
            cm.__exit__(None, None, None)

    nc.finalize()
    return nc


def make_in_maps(x, context, Wq, Wk, Wv, Wo, bo):
    """Host-side sharding + layout prep: transpose and cast to bf16."""
    import ml_dtypes
    bf16 = ml_dtypes.bfloat16

    x = np.asarray(x, np.float32)
    context = np.asarray(context, np.float32)
    wq = np.asarray(Wq, np.float32).astype(bf16)
    wk = np.asarray(Wk, np.float32).astype(bf16)
    wv = np.asarray(Wv, np.float32).astype(bf16)
    wo = np.asarray(Wo, np.float32).astype(bf16)
    bo = np.asarray(bo, np.float32).reshape(1, QD).astype(bf16)
    eye = np.eye(128, dtype=np.float32).astype(bf16)

    in_maps = []
    for c in range(N_CORES):
        b, half = c // 2, c % 2
        xs = x[b, half * NQL:(half + 1) * NQL, :]           # [2048, 1024]
        in_maps.append({
            "xT": np.ascontiguousarray(xs.T).astype(bf16),   # [1024, 2048]
            "ctxT": np.ascontiguousarray(context[b].T).astype(bf16),
            "wq": wq, "wk": wk, "wv": wv, "wo": wo, "bo": bo, "eye": eye,
        })
    return in_maps


_NC_CACHE = {}


def kernel(x, context, Wq, Wk, Wv, Wo, bo, _trace=False):
    import sys
    if "/opt/trn_rl_repo" not in sys.path:
        sys.path.insert(0, "/opt/trn_rl_repo")
    from concourse.bass_utils import run_bass_kernel_spmd

    if "nc" not in _NC_CACHE:
        _NC_CACHE["nc"] = build_bass()
    nc = _NC_CACHE["nc"]

    in_maps = make_in_maps(x, context, Wq, Wk, Wv, Wo, bo)
    res = run_bass_kernel_spmd(
        nc, in_maps, core_ids=list(range(N_CORES)), trace=_trace)

    out = np.empty((B, NQ, QD), np.float32)
    for c in range(N_CORES):
        b, half = c // 2, c % 2
        out[b, half * NQL:(half + 1) * NQL, :] = res.results[c]["out"]
    if _trace:
        return out, res
    return out


# revision 17
# speedup vs baseline: 1.1943x; 1.1943x over previous
"""Cross-attention Trainium2 kernel, 8-core data-parallel.

Problem (hardcoded): B=4, NQ=4096, NK=1024, QD=1024, CD=768, H=16, HD=64.
  out = softmax((x@Wq) @ (ctx@Wk)^T / sqrt(HD)) @ (ctx@Wv) @ Wo + bo

Sharding: pure data-parallel. 8 cores = 4 batches x 2 NQ-halves of 2048
query rows. Each core redundantly computes K/V projections for its batch
(cheap) and needs no collectives.

v2 design (from perfetto analysis of v1):
 - The scalar ACT engine (exp over 33.5M scores/core at 1 elem/cycle/lane
   @1.2GHz + ~300cyc/instr overhead) is the binding engine of the
   attention phase.  Design goal: ACT 100% duty, everything else hidden.
 - exp instructions enlarged to N=2048/1024 via PSUM staging slots
   A=[128,2,1024] f32 (4 banks) + B=[128,1024] (2 banks); per head-pair
   the 8 key-chunks map kc{0,1}->A, kc2->B, kc{3,4}->A, kc5->B,
   kc{6,7}->A giving 5 exps (3x2048 + 2x1024) instead of 8x1024.
 - One shared PSUM accumulator ring ACC=[128,2,512] f32 (2 banks) serves
   attn@V groups, out-projection, Q/K/V-projection groups.
 - 4 query-tiles of 512.  PE stream per (tile, head-pair): score
   pair-matmuls (tile_position row-split => the two 64-contraction
   matmuls run concurrently), the PREVIOUS head-pair's attn@V (its exp
   outputs are complete; etp is a 2-deep ring over head-pairs), and
   filler matmul groups (V-projection during tile0, out-projection of
   tile t-1, Q-projection of tile t+1) slotted between score pairs so
   the PE always has runnable work while exp drains.
 - Scalar engine does nothing but exp during attention; PSUM->SBUF
   copies go to vector/gpsimd.
"""

import numpy as np

B, NQ, NK = 4, 4096, 1024
QD, CD, H = 1024, 768, 16
HD = QD // H
SCALE = HD ** -0.5
NQL = NQ // 2          # query rows per core
N_CORES = 8
W = 512                # query tile width
NT = NQL // W          # 4 query tiles
KC_Q = QD // 128       # 8
KC_C = CD // 128       # 6
NKC = NK // 128        # 8
HP = H // 2            # 8 head pairs


def build_bass():
    """Build the per-core Bass graph (SPMD, identical on all 8 cores)."""
    import concourse.bass as bass
    import concourse.tile as tile
    from concourse import bacc, mybir

    f32 = mybir.dt.float32
    bf16 = mybir.dt.bfloat16
    EXP = mybir.ActivationFunctionType.Exp

    nc = bacc.Bacc()

    xT_h = nc.dram_tensor("xT", (QD, NQL), bf16, kind="ExternalInput")
    ctxT_h = nc.dram_tensor("ctxT", (CD, NK), bf16, kind="ExternalInput")
    wq_h = nc.dram_tensor("wq", (QD, QD), bf16, kind="ExternalInput")
    wk_h = nc.dram_tensor("wk", (CD, QD), bf16, kind="ExternalInput")
    wv_h = nc.dram_tensor("wv", (CD, QD), bf16, kind="ExternalInput")
    wo_h = nc.dram_tensor("wo", (QD, QD), bf16, kind="ExternalInput")
    bo_h = nc.dram_tensor("bo", (1, QD), bf16, kind="ExternalInput")
    eye_h = nc.dram_tensor("eye", (128, 128), bf16, kind="ExternalInput")
    out_h = nc.dram_tensor("out", (NQL, QD), f32, kind="ExternalOutput")

    xT_d = xT_h[:].rearrange("(c p) n -> p c n", p=128)       # [128, 8, 2048]
    ctxT_d = ctxT_h[:].rearrange("(c p) n -> p c n", p=128)   # [128, 6, 1024]
    wq_d = wq_h[:].rearrange("(c p) m -> p c m", p=128)
    wk_d = wk_h[:].rearrange("(c p) m -> p c m", p=128)
    wv_d = wv_h[:].rearrange("(c p) m -> p c m", p=128)
    wo_d = wo_h[:].rearrange("(c p) m -> p c m", p=128)
    out_d = out_h[:].rearrange("(t p) n -> p t n", p=128)     # [128, 16, 1024]

    with tile.TileContext(nc) as tc:
        _cms = []

        def open_pool(**kw):
            cm = tc.tile_pool(**kw)
            _cms.append(cm)
            return cm.__enter__()

        pp = open_pool(name="persist", bufs=1)
        # ---- persistent SBUF tiles
        qt_sb = pp.tile([128, KC_Q, 2, W], bf16)       # QT ring (per-tile)
        kt_sb = pp.tile([128, KC_Q, NK], bf16)         # KT   2 MB
        vp_sb = pp.tile([128, NKC, H, HD + 1], bf16)   # V'   2.08 MB
        attnT_sb = pp.tile([128, KC_Q, 2, W], bf16)    # attn ring (per-tile)
        wo_sb = pp.tile([128, KC_Q, QD], bf16)         # Wo   2 MB
        wq_sb = pp.tile([128, KC_Q, QD], bf16)         # Wq   2 MB
        # exp outputs: ring of 2 hp-buffers: slot index = (hp%2)*8 + kc
        etp = pp.tile([128, 16, 2, W], bf16)           # 4.2 MB
        bo_bc = pp.tile([128, QD], bf16)
        ones_sb = pp.tile([1, 128], bf16)
        eye_sb = pp.tile([128, 128], bf16)

        # ---- PSUM (8 banks): A 4 + Bp 2 + ACC ring 2
        psS = open_pool(name="psS", bufs=1, space=bass.MemorySpace.PSUM)
        A = psS.tile([128, 2, 1024], f32)
        Bp = psS.tile([128, 1024], f32)
        accp = open_pool(name="accp", bufs=2, space=bass.MemorySpace.PSUM)

        # ---- small SBUF pools
        py = open_pool(name="yout", bufs=3)
        prs = open_pool(name="rsmall", bufs=2)
        pxq = open_pool(name="xq", bufs=2)
        pkv = open_pool(name="kvin", bufs=1)

        ctxT_sb = pkv.tile([128, KC_C, NK], bf16)      # 1.5 MB
        wk_sb = pkv.tile([128, KC_C, QD], bf16)        # 1.5 MB
        wv_sb = pkv.tile([128, KC_C, QD], bf16)        # 1.5 MB
        bo_sb = pkv.tile([1, QD], bf16)

        # copy engines for PSUM->SBUF moves; scalar only in the prefix
        _cp_idx = [0]

        def copy_out(dst, src, in_attention):
            # gpsimd cannot read PSUM; scalar must stay free during attention
            engs = (nc.vector,) if in_attention else (nc.vector, nc.scalar)
            eng = engs[_cp_idx[0] % len(engs)]
            _cp_idx[0] += 1
            if eng is nc.scalar:
                eng.copy(dst, src)
            else:
                eng.tensor_copy(dst, src)

        # ---------------- prefix ----------------
        nc.sync.dma_start(ctxT_sb[:, :, 0:512], ctxT_d[:, :, 0:512])
        nc.sync.dma_start(wk_sb[:, :, 0:256], wk_d[:, :, 0:256])
        nc.sync.dma_start(bo_sb[:], bo_h[:])
        nc.sync.dma_start(ctxT_sb[:, :, 512:1024], ctxT_d[:, :, 512:1024])
        nc.sync.dma_start(wk_sb[:, :, 256:1024], wk_d[:, :, 256:1024])

        nc.sync.dma_start(eye_sb[:], eye_h[:])
        xt0 = pxq.tile([128, KC_Q, W], bf16, tag="xt", name="xt0")

        nc.vector.memset(ones_sb[:], 1.0)
        nc.vector.memset(vp_sb[:, :, :, HD], 1.0)    # ones column per head
        for no in range(2):
            ps = accp.tile([128, 512], f32, tag="acc", name="psb")
            nc.tensor.matmul(ps[:], ones_sb[:],
                             bo_sb[0:1, no * 512:(no + 1) * 512],
                             start=True, stop=True)
            nc.scalar.copy(bo_bc[:, no * 512:(no + 1) * 512], ps[:])

        # ---- K projection (all keys; needed before any scores)
        for mo in range(KC_Q):
            for nk in range(2):
                ps = accp.tile([128, 512], f32, tag="acc", name="psk")
                for c in range(KC_C):
                    nc.tensor.matmul(
                        ps[:],
                        wk_sb[:, c, mo * 128:(mo + 1) * 128],
                        ctxT_sb[:, c, nk * 512:(nk + 1) * 512],
                        start=(c == 0), stop=(c == KC_C - 1),
                    )
                copy_out(kt_sb[:, mo, nk * 512:(nk + 1) * 512], ps[:], False)
            if mo == 0:
                nc.sync.dma_start(wv_sb[:], wv_d)
                nc.sync.dma_start(xt0[:], xT_d[:, :, 0:W])
                nc.sync.dma_start(wq_sb[:], wq_d)

        def v_proj(ko, nv, in_attention):
            ps = accp.tile([128, 512], f32, tag="acc", name="psv")
            for c in range(KC_C):
                nc.tensor.matmul(
                    ps[:],
                    ctxT_sb[:, c, ko * 128:(ko + 1) * 128],
                    wv_sb[:, c, nv * 512:(nv + 1) * 512],
                    start=(c == 0), stop=(c == KC_C - 1),
                )
            copy_out(
                vp_sb[:, ko, nv * 8:(nv + 1) * 8, 0:HD],
                ps[:].rearrange("p (h d) -> p h d", h=8), in_attention)

        # ---- V projection for heads 0-7 (needed by tile0's early attn@V)
        for ko in range(NKC):
            v_proj(ko, 0, False)

        # ---- Q projection for tile 0
        for mo in range(KC_Q):
            ps = accp.tile([128, 512], f32, tag="acc", name="psq")
            for c in range(KC_Q):
                nc.tensor.matmul(
                    ps[:],
                    wq_sb[:, c, mo * 128:(mo + 1) * 128],
                    xt0[:, c, :],
                    start=(c == 0), stop=(c == KC_Q - 1),
                )
            copy_out(qt_sb[:, mo, 0, :], ps[:], False)

        nc.sync.dma_start(wo_sb[:], wo_d)

        # ---------- filler generators (yield once per matmul group) ----
        # V heads 8-15: drained during tile0 hp0-3, strictly before any
        # attn@V of hp>=4 (which is emitted at hp>=5) reads them.
        def v_group():
            for ko in range(NKC):
                v_proj(ko, 1, True)
                yield

        def q_group(t, xt):
            for mo in range(KC_Q):
                ps = accp.tile([128, 512], f32, tag="acc", name="psq2")
                for c in range(KC_Q):
                    nc.tensor.matmul(
                        ps[:],
                        wq_sb[:, c, mo * 128:(mo + 1) * 128],
                        xt[:, c, :],
                        start=(c == 0), stop=(c == KC_Q - 1),
                    )
                copy_out(qt_sb[:, mo, t % 2, :], ps[:], True)
                yield

        def y_group(t):
            for lo in range(4):
                mo = t * 4 + lo
                for no in range(2):
                    ps = accp.tile([128, 512], f32, tag="acc", name="psy")
                    for c in range(KC_Q):
                        nc.tensor.matmul(
                            ps[:],
                            attnT_sb[:, c, t % 2, lo * 128:(lo + 1) * 128],
                            wo_sb[:, c, no * 512:(no + 1) * 512],
                            start=(c == 0), stop=(c == KC_Q - 1),
                        )
                    y = py.tile([128, 512], f32, tag="y")
                    nc.vector.tensor_add(
                        y[:], ps[:], bo_bc[:, no * 512:(no + 1) * 512])
                    nc.sync.dma_start(
                        out_d[:, mo, no * 512:(no + 1) * 512], y[:])
                    yield

        # ---------------- main attention loop ----------------
        pst = open_pool(name="stg", bufs=12)
        _stages = {}

        def attn_vO(t, hp, qcs):
            """attn@V in O[q,d] form + normalize into SBUF stage tiles.

            One PSUM slot holds both heads: O[:,0:65]=h0 (col 64 = softmax
            denominator), O[:,65:130]=h1.  Normalize with a per-partition
            reciprocal + tensor_scalar into stage[q,d] (bf16).  The PE
            transpose back to attnT's [d,q] layout happens one head-pair
            later (attn_vT) so the PE never waits on this DVE chain.
            """
            r = hp % 2
            for qc in qcs:
                O = accp.tile([128, 512], f32, tag="acc", name="O")
                for h_i in (0, 1):
                    h = 2 * hp + h_i
                    dst = O[:, h_i * 65:h_i * 65 + 65]
                    for kc in range(NKC):
                        nc.tensor.matmul(
                            dst,
                            etp[:, r * 8 + kc, h_i, qc * 128:(qc + 1) * 128],
                            vp_sb[:, kc, h, :],
                            start=(kc == 0), stop=(kc == NKC - 1),
                        )
                stage = pst.tile([128, 128], bf16, tag="st")
                for h_i in (0, 1):
                    rcp = prs.tile([128, 1], f32, tag="rcp")
                    nc.vector.reciprocal_approx_fast(
                        rcp[:], O[:, h_i * 65 + 64:h_i * 65 + 65])
                    nc.vector.tensor_scalar_mul(
                        stage[:, h_i * 64:(h_i + 1) * 64],
                        O[:, h_i * 65:h_i * 65 + 64], rcp[:])
                _stages[(t, hp, qc)] = stage

        def attn_vT(t, hp, qcs):
            for qc in qcs:
                stage = _stages.pop((t, hp, qc))
                tp = accp.tile([128, 512], f32, tag="acc", name="tp")
                tp16 = tp[:, 0:64].bitcast(bf16)
                nc.tensor.transpose(tp16, stage[:], eye_sb[:])
                nc.vector.tensor_copy(
                    attnT_sb[:, hp, t % 2, qc * 128:(qc + 1) * 128], tp16)

        pending = None          # (t, hp) owed attn@V O/normalize
        pending2 = None         # (t, hp) owed transposes
        fillers = []

        def take_filler(n):
            # drain sequentially: first generator until exhausted, then next
            for _ in range(n):
                while fillers:
                    if next(fillers[0], "DONE") == "DONE":
                        fillers.pop(0)
                        continue
                    break

        # y(t-1) must not drain at hp0: its c=7 matmul needs the transpose
        # of (t-1, hp7) which is only emitted at hp1.
        N_FILL = [0, 2, 2, 3, 3, 2, 2, 2]

        for t in range(NT):
            if t == 0:
                fillers.append(v_group())
            else:
                fillers.append(y_group(t - 1))
            if t + 1 < NT:
                xt = pxq.tile([128, KC_Q, W], bf16, tag="xt",
                              name=f"xt{t + 1}")
                nc.sync.dma_start(xt[:], xT_d[:, :, (t + 1) * W:(t + 2) * W])
                fillers.append(q_group(t + 1, xt))

            for hp in range(HP):
                r = hp % 2

                def spair(kc, dst_h0, dst_h1):
                    ks = slice(kc * 128, (kc + 1) * 128)
                    nc.tensor.matmul(
                        dst_h0, kt_sb[0:64, hp, ks],
                        qt_sb[0:64, hp, t % 2, :],
                        start=True, stop=True, tile_position=(0, 0),
                    )
                    nc.tensor.matmul(
                        dst_h1, kt_sb[64:128, hp, ks],
                        qt_sb[64:128, hp, t % 2, :],
                        start=True, stop=True, tile_position=(64, 0),
                    )

                def exp_a(kc):
                    nc.scalar.activation(
                        etp[:, r * 8 + kc:r * 8 + kc + 2, :, :],
                        A[:], EXP, scale=SCALE)

                def exp_b(kc):
                    nc.scalar.activation(
                        etp[:, r * 8 + kc:r * 8 + kc + 1, :, :],
                        Bp[:], EXP, scale=SCALE)

                # kc pattern: {0,1}->A, 2->B, {3,4}->A, 5->B, {6,7}->A
                spair(0, A[:, 0, 0:512], A[:, 0, 512:1024])
                spair(1, A[:, 1, 0:512], A[:, 1, 512:1024])
                exp_a(0)
                spair(2, Bp[:, 0:512], Bp[:, 512:1024])
                exp_b(2)
                if pending2 is not None:
                    attn_vT(*pending2, (0, 1, 2, 3))
                if pending is not None:
                    attn_vO(*pending, (0, 1))
                spair(3, A[:, 0, 0:512], A[:, 0, 512:1024])
                spair(4, A[:, 1, 0:512], A[:, 1, 512:1024])
                exp_a(3)
                if pending is not None:
                    attn_vO(*pending, (2,))
                spair(5, Bp[:, 0:512], Bp[:, 512:1024])
                exp_b(5)
                if pending is not None:
                    attn_vO(*pending, (3,))
                spair(6, A[:, 0, 0:512], A[:, 0, 512:1024])
                spair(7, A[:, 1, 0:512], A[:, 1, 512:1024])
                exp_a(6)
                take_filler(N_FILL[hp])
                pending2 = pending
                pending = (t, hp)

        # drain: last two head-pairs' attn@V stages + last y tile
        attn_vT(*pending2, (0, 1))
        attn_vO(*pending, (0, 1, 2, 3))
        attn_vT(*pending2, (2, 3))
        take_filler(1000)
        attn_vT(*pending, (0, 1, 2, 3))
        for _ in y_group(NT - 1):
            pass

        for cm in reversed(_cms):
            cm.__exit__(None, None, None)

    nc.finalize()
    return nc


def make_in_maps(x, context, Wq, Wk, Wv, Wo, bo):
    """Host-side sharding + layout prep: transpose and cast to bf16."""
    import ml_dtypes
    bf16 = ml_dtypes.bfloat16

    x = np.asarray(x, np.float32)
    context = np.asarray(context, np.float32)
    wq = np.asarray(Wq, np.float32).astype(bf16)
    wk = np.asarray(Wk, np.float32).astype(bf16)
    wv = np.asarray(Wv, np.float32).astype(bf16)
    wo = np.asarray(Wo, np.float32).astype(bf16)
    bo = np.asarray(bo, np.float32).reshape(1, QD).astype(bf16)
    eye = np.eye(128, dtype=np.float32).astype(bf16)

    in_maps = []
    for c in range(N_CORES):
        b, half = c // 2, c % 2
        xs = x[b, half * NQL:(half + 1) * NQL, :]           # [2048, 1024]
        in_maps.append({
            "xT": np.ascontiguousarray(xs.T).astype(bf16),   # [1024, 2048]
            "ctxT": np.ascontiguousarray(context[b].T).astype(bf16),
            "wq": wq, "wk": wk, "wv": wv, "wo": wo, "bo": bo, "eye": eye,
        })
    return in_maps


_NC_CACHE = {}


def kernel(x, context, Wq, Wk, Wv, Wo, bo, _trace=False):
    import sys
    if "/opt/trn_rl_repo" not in sys.path:
        sys.path.insert(0, "/opt/trn_rl_repo")
    from concourse.bass_utils import run_bass_kernel_spmd

    if "nc" not in _NC_CACHE:
        _NC_CACHE["nc"] = build_bass()
    nc = _NC_CACHE["nc"]

    in_maps = make_in_maps(x, context, Wq, Wk, Wv, Wo, bo)
    res = run_bass_kernel_spmd(
        nc, in_maps, core_ids=list(range(N_CORES)), trace=_trace)

    out = np.empty((B, NQ, QD), np.float32)
    for c in range(N_CORES):
        b, half = c // 2, c % 2
        out[b, half * NQL:(half + 1) * NQL, :] = res.results[c]["out"]
    if _trace:
        return out, res
    return out
